# revision 19
# baseline (speedup 1.0000x reference)
"""SAGAN-style attention block on 8 TRN2 NeuronCores, data-parallel over batch.

Per core (one batch b): x_b [C=256, N=4096] f32.
  q = Wq x + bq  [32, N];  k = Wk x + bk  [32, N]
  S = q^T k  [N, N];  attn = softmax(S, axis=0)  (column softmax over i)
  out = gamma * (v @ attn) + x,  v = Wv x + bv

Device algorithm (bf16 matmuls, f32 PSUM accumulation):
  - no max-subtraction in softmax: |S| < ~50 empirically, exp() fits bf16
  - bv folded into the residual: out = gamma*(v0@E)/s + (x + gamma*bv)
  - qk emitted as 4-way row-tiled waves (tile_position) into a [P,4,JT]
    PSUM tile; exp activations read FD=2048 (4 banks) at a time
  - colsum via 4-way col-tiled ones[128,32] matmuls reading E directly
    (partials in 4 partition groups), then one broadcast matmul with a
    1/(32*gamma) stationary -> s/gamma on all 128 partitions
  - v@E matmuls split into M=64 col-tile pairs so each LDWEIGHTS (64
    cols) can hide behind the concurrent matmul on the other col group
"""

import numpy as np
import ml_dtypes

import concourse.bass as bass
import concourse.mybir as mybir
from concourse import bacc, tile
from concourse.bass import ds
from concourse.bass_utils import run_bass_kernel_spmd

F32 = mybir.dt.float32
BF16 = mybir.dt.bfloat16
FP8 = mybir.dt.float8e4
AF = mybir.ActivationFunctionType
ALU = mybir.AluOpType

B, C, N = 8, 256, 4096
C8 = 32
P = 128
JT = 512          # j-tile width
NJT = N // JT     # 8 j-tiles
NKC = N // P      # 32 i/k chunks of 128

_cache = {}


def _build_nc():
    nc = bacc.Bacc("TRN2", target_bir_lowering=False, debug=False, num_devices=B)

    x16_d = nc.dram_tensor("x16", [C, N], BF16, kind="ExternalInput").ap()
    wq_d = nc.dram_tensor("wq", [P, 2, P], BF16, kind="ExternalInput").ap()
    wk_d = nc.dram_tensor("wk", [P, 2, P], BF16, kind="ExternalInput").ap()
    wvt_d = nc.dram_tensor("wvt", [P, 2, C], BF16, kind="ExternalInput").ap()
    bq_d = nc.dram_tensor("bq", [P, 1], F32, kind="ExternalInput").ap()
    bk_d = nc.dram_tensor("bk", [P, 1], F32, kind="ExternalInput").ap()
    gbv_d = nc.dram_tensor("gbv", [P, 2], F32, kind="ExternalInput").ap()
    bones_d = nc.dram_tensor("bones", [P, P], BF16, kind="ExternalInput").ap()
    out_d = nc.dram_tensor("out", [C, N], F32, kind="ExternalOutput").ap()

    out_view = out_d.rearrange("(o p) n -> p o n", p=P)
    x16_view = x16_d.rearrange("(o p) n -> p o n", p=P)

    with tile.TileContext(nc) as tc:
        with (
            tc.tile_pool(name="const", bufs=1) as cpool,
            tc.tile_pool(name="big", bufs=1) as bigpool,
            tc.tile_pool(name="epool", bufs=2) as epool,
            tc.tile_pool(name="work", bufs=2) as wpool,
            tc.tile_pool(name="psA", bufs=1, space="PSUM") as psA,
            tc.tile_pool(name="psU", bufs=2, space="PSUM") as psU,
            tc.tile_pool(name="psS", bufs=1, space="PSUM") as psS,
            tc.tile_pool(name="psR", bufs=1, space="PSUM") as psR,
        ):
            # ---- big input first: x16 (2 MB) is the ONLY big input DMA.
            # The residual is added in bf16 (rel rms ~1e-3, far under the
            # 2e-2 gate) with gamma*bv folded into the fused normalize op.
            # First chunks are small so the projections can start early.
            x16_sb = bigpool.tile([P, 2, N], BF16, tag="x16")
            x16_chunks = [512, 512, 1024, 1024, 1024]
            off = 0
            for w_ch in x16_chunks:
                nc.sync.dma_start(
                    out=x16_sb[:, :, ds(off, w_ch)],
                    in_=x16_view[:, :, ds(off, w_ch)],
                )
                off += w_ch

            # ---- constants ----
            wq_sb = cpool.tile([P, 2, P], BF16, tag="wq")
            nc.sync.dma_start(out=wq_sb[:], in_=wq_d)
            wk_sb = cpool.tile([P, 2, P], BF16, tag="wk")
            nc.sync.dma_start(out=wk_sb[:], in_=wk_d)
            bq_sb = cpool.tile([P, 1], F32, tag="bq")
            nc.sync.dma_start(out=bq_sb[:], in_=bq_d)
            bk_sb = cpool.tile([P, 1], F32, tag="bk")
            nc.sync.dma_start(out=bk_sb[:], in_=bk_d)
            wvt_sb = cpool.tile([P, 2, C], BF16, tag="wvt")
            nc.sync.dma_start(out=wvt_sb[:], in_=wvt_d)
            gbv_sb = cpool.tile([P, 2], F32, tag="gbv")
            nc.sync.dma_start(out=gbv_sb[:], in_=gbv_d)
            bones_sb = cpool.tile([P, P], BF16, tag="bones")
            nc.sync.dma_start(out=bones_sb[:], in_=bones_d)
            ones32_sb = cpool.tile([P, C8], BF16, tag="ones32")
            nc.vector.memset(ones32_sb[:], 1.0)

            # preload the exp table during the DMA wait
            dummy = cpool.tile([P, 1], BF16, tag="dummy")
            nc.scalar.activation(dummy[:], bq_sb[:], AF.Exp)

            q_sb = bigpool.tile([P, N], BF16, tag="q")
            k_sb = bigpool.tile([P, N], BF16, tag="k")
            vt_sb = bigpool.tile([P, NKC, C], BF16, tag="vt")

            # ---- q/k projections, chunk-major so qk wave 0 can start after
            # the first n-chunk: cast(2g), cast(2g+1), q-proj(g), k-proj(g)
            # out[m, n] = sum_c W_rep[c, m] * x16[c, n]  (M=128: 4 replicas)
            proj_ps = psA.tile([P, 4, JT], F32, tag="qk", name="proj_ps")
            for g in range(4):
                for gi, (w_sb, b_sb, dst) in enumerate(
                    ((wq_sb, bq_sb, q_sb), (wk_sb, bk_sb, k_sb))
                ):
                    sl = proj_ps[:, ds(2 * gi, 2), :]
                    for t2 in range(2):
                        nch = 2 * g + t2
                        for kc in range(2):
                            nc.tensor.matmul(
                                sl[:, t2, :],
                                w_sb[:, kc, :],
                                x16_sb[:, kc, ds(nch * JT, JT)],
                                start=(kc == 0),
                                stop=(kc == 1),
                            )
                    nc.vector.tensor_scalar(
                        out=dst[:, ds(g * 2 * JT, 2 * JT)],
                        in0=sl[:].rearrange("p a b -> p (a b)"),
                        scalar1=b_sb[:, :],
                        scalar2=None,
                        op0=ALU.add,
                    )

            # vT projection emitted inside superstep 0 (interleaved below),
            # using the psU buffers (idle until superstep 1):
            # out[n, c] = sum_c' x16[c', n] * WvT[c', c]
            def emit_vt_group(g):
                sl = psU.tile([P, 2, C], F32, tag="u", name=f"vtp_{g}")
                for t2 in range(2):
                    nck = 2 * g + t2
                    for kc in range(2):
                        nc.tensor.matmul(
                            sl[:, t2, :],
                            x16_sb[:, kc, ds(nck * P, P)],
                            wvt_sb[:, kc, :],
                            start=(kc == 0),
                            stop=(kc == 1),
                        )
                nc.vector.tensor_copy(
                    out=vt_sb[:, ds(2 * g, 2), :], in_=sl[:]
                )

            # ---- main software-pipelined loop over j-tiles ----
            # Per superstep t: qk+exp waves for tile t, v@E for tile t-1.
            # Colsum pair-waves lag 2 waves behind their acts (so the DVE
            # pair-sum latency never stalls the TensorE queue); the last 4
            # pairs of tile t and its finalize (copy/bcast/recip) are spread
            # over waves 0-2 of superstep t+1.
            e_tiles = {}
            ep_tiles = {}
            eq_tiles = {}
            s_ps_t = {}
            r_tiles = {}
            u_ps_t = {}

            for step in range(NJT + 1):
                t_prev = step - 1

                if step < NJT:
                    e_tiles[step] = epool.tile(
                        [P, NKC, JT], BF16, tag="E", name=f"E_{step}"
                    )
                    ep_tiles[step] = epool.tile(
                        [P, NKC // 2, JT], BF16, tag="Ep", name=f"Ep_{step}",
                        bufs=1,
                    )
                    eq_tiles[step] = epool.tile(
                        [P, NJT, JT], BF16, tag="Eq", name=f"Eq_{step}"
                    )
                    s_ps_t[step] = psS.tile([P, JT], F32, tag="s", name=f"s_{step}")
                if step >= 1:
                    u_ps_t[t_prev] = [
                        psU.tile([P, JT], F32, tag="u", name=f"u_{t_prev}_{m}")
                        for m in range(2)
                    ]

                js = ds(step * JT, JT) if step < NJT else None
                e_cur = e_tiles.get(step)
                ep_cur = ep_tiles.get(step)
                e_prev = e_tiles.get(t_prev)
                qk_ps = (
                    psA.tile([P, 4, JT], F32, tag="qk", name=f"qk_{step}")
                    if step < NJT else None
                )

                def emit_cs_pairwave(t, wp):
                    # 1 col-tiled ones matmul on the quad sum of wave wp
                    cg = wp % 4
                    nc.tensor.matmul(
                        s_ps_t[t][ds(32 * cg, 32), :],
                        ones32_sb[:],
                        eq_tiles[t][:, wp, :],
                        start=(wp < 4),
                        stop=(wp >= NJT - 4),
                        tile_position=(0, 32 * cg),
                    )

                def emit_finalize_piece(t, w):
                    # spread tile t's colsum tail + finalize over waves 0-2
                    if w == 0:
                        emit_cs_pairwave(t, NJT - 2)
                    elif w == 1:
                        emit_cs_pairwave(t, NJT - 1)
                        s128 = wpool.tile([P, JT], BF16, tag="s128")
                        nc.vector.tensor_copy(out=s128[:], in_=s_ps_t[t][:])
                        s_ps_t[t] = s128
                    elif w == 2:
                        rps = psR.tile([P, JT], F32, tag="r")
                        nc.tensor.matmul(rps[:], bones_sb[:], s_ps_t[t][:],
                                         start=True, stop=True)
                        r_sb = wpool.tile([P, JT], F32, tag="r")
                        nc.vector.reciprocal_approx_fast(out=r_sb[:], in_=rps[:])
                        r_tiles[t] = r_sb

                for w in range(NJT):
                    # v@E slots for tile t-1 (8 slots = 16 M=64 matmuls)
                    if step >= 1:
                        for s in range(8):
                            # m-major: all m=0 chunks then m=1, so each u
                            # bank's final write lands mid-wave and the next
                            # superstep's start=True matmuls never stall on
                            # the normalize muls
                            m, kc = s // 4, 4 * w + s % 4
                            for h in range(2):
                                nc.tensor.matmul(
                                    u_ps_t[t_prev][m][ds(64 * h, 64), :],
                                    vt_sb[:, kc, ds(128 * m + 64 * h, 64)],
                                    e_prev[:, kc, :],
                                    start=(kc == 0),
                                    stop=(kc == NKC - 1),
                                    tile_position=(0, 64 * h),
                                )
                    # qk wave w: 4 row-tiled matmuls (rows cycle 0..3),
                    # directly after the v@E slots: the act(w-1) -> qk(w)
                    # -> act(w) chain is the pacing loop
                    if step < NJT:
                        for r in range(4):
                            ic = 4 * w + r
                            nc.tensor.matmul(
                                qk_ps[:, r, :],
                                q_sb[32 * r: 32 * (r + 1), ds(ic * P, P)],
                                k_sb[32 * r: 32 * (r + 1), js],
                                start=True,
                                stop=True,
                                tile_position=(32 * r, 0),
                            )
                    # previous tile's colsum tail + finalize (waves 0-2)
                    if step >= 1 and w <= 2:
                        emit_finalize_piece(t_prev, w)
                    # colsum pair-waves for tile t, lagging 2 waves
                    if step < NJT and w >= 2:
                        emit_cs_pairwave(step, w - 2)
                    # normalize tile t-1 late in the superstep: free the u
                    # banks (muls first), then residual-add and store
                    if step >= 1 and w == NJT - 1:
                        js_prev = ds(t_prev * JT, JT)
                        r_sb = r_tiles[t_prev]
                        outt = wpool.tile([P, 2, JT], F32, tag="outt")
                        tmps = []
                        for m in range(2):
                            tmp = wpool.tile([P, JT], F32, tag="tmp")
                            nc.vector.tensor_mul(
                                out=tmp[:], in0=u_ps_t[t_prev][m][:],
                                in1=r_sb[:],
                            )
                            tmps.append(tmp)
                        for m in range(2):
                            nc.vector.scalar_tensor_tensor(
                                out=outt[:, m, :],
                                in0=tmps[m][:],
                                scalar=gbv_sb[:, m, None],
                                in1=x16_sb[:, m, js_prev],
                                op0=ALU.add,
                                op1=ALU.add,
                            )
                        nc.sync.dma_start(
                            out=out_view[:, :, js_prev], in_=outt[:]
                        )

                    # activation over all 4 banks (FD=2048)
                    if step < NJT:
                        nc.scalar.activation(
                            e_cur[:, ds(4 * w, 4), :], qk_ps[:], AF.Exp
                        )
                        # chunk-pair + quad sums (DVE): quad w lands in
                        # ep[w] = sum of the wave's 4 chunks
                        nc.vector.tensor_add(
                            out=ep_cur[:, 2 * w, :],
                            in0=e_cur[:, 4 * w, :],
                            in1=e_cur[:, 4 * w + 1, :],
                        )
                        nc.vector.tensor_add(
                            out=ep_cur[:, 2 * w + 1, :],
                            in0=e_cur[:, 4 * w + 2, :],
                            in1=e_cur[:, 4 * w + 3, :],
                        )
                        nc.vector.tensor_add(
                            out=eq_tiles[step][:, w, :],
                            in0=ep_cur[:, 2 * w, :],
                            in1=ep_cur[:, 2 * w + 1, :],
                        )
                    # superstep 0 filler: vT projection
                    if step == 0:
                        emit_vt_group(2 * w)
                        emit_vt_group(2 * w + 1)
    nc.compile()
    return nc


def _prep_inputs(x, Wq, bq, Wk, bk, Wv, bv, gamma):
    x = np.asarray(x, dtype=np.float32)
    Wq = np.asarray(Wq, dtype=np.float32)
    bq = np.asarray(bq, dtype=np.float32)
    Wk = np.asarray(Wk, dtype=np.float32)
    bk = np.asarray(bk, dtype=np.float32)
    Wv = np.asarray(Wv, dtype=np.float32)
    bv = np.asarray(bv, dtype=np.float32)
    g = float(np.asarray(gamma))

    bf = ml_dtypes.bfloat16
    # WqT replicated 4x along M so q lands replicated across 4x32 partitions
    wq_rep = np.tile(Wq.T, (1, 4)).reshape(2, P, P).transpose(1, 0, 2)
    wk_rep = np.tile(Wk.T, (1, 4)).reshape(2, P, P).transpose(1, 0, 2)
    wvt = Wv.T.reshape(2, P, C).transpose(1, 0, 2)
    bq_rep = np.tile(bq, 4)[:, None].astype(np.float32)
    bk_rep = np.tile(bk, 4)[:, None].astype(np.float32)
    gbv = (g * bv).reshape(2, P).T.copy().astype(np.float32)
    inv32g = 1.0 / (32.0 * g) if g != 0.0 else 0.0
    bones = np.full((P, P), inv32g, dtype=np.float32)

    xf = x.reshape(B, C, N)
    x16 = xf.astype(bf)

    shared = {
        "wq": np.ascontiguousarray(wq_rep.astype(bf)),
        "wk": np.ascontiguousarray(wk_rep.astype(bf)),
        "wvt": np.ascontiguousarray(wvt.astype(bf)),
        "bq": bq_rep,
        "bk": bk_rep,
        "gbv": gbv,
        "bones": bones.astype(bf),
    }
    in_maps = []
    for b in range(B):
        m = dict(shared)
        m["x16"] = np.ascontiguousarray(x16[b])
        in_maps.append(m)
    return in_maps


def _get_nc():
    if "nc" not in _cache:
        _cache["nc"] = _build_nc()
    return _cache["nc"]


def _install_neff_cache():
    """Cache compiled NEFFs by BIR hash: the bass_exec path skips the
    regular neuron compile cache, costing ~10min of walrus per process."""
    import hashlib
    import pathlib
    import shutil

    from concourse import bass2jax as b2j

    if getattr(b2j, "_ant_neff_cache_installed", False):
        return
    orig = b2j.compile_bir_kernel
    cache_dir = pathlib.Path("/root/.neuron-compile-cache/bass_neff")
    try:
        cache_dir.mkdir(parents=True, exist_ok=True)
    except OSError:
        return

    def cached(bir_json, tmpdir, neff_name="file.neff"):
        raw = bir_json if isinstance(bir_json, bytes) else bir_json.encode()
        h = hashlib.sha256(raw).hexdigest()
        hit = cache_dir / f"{h}.neff"
        if hit.exists():
            sg = pathlib.Path(tmpdir) / "sg00"
            sg.mkdir(parents=True, exist_ok=True)
            out = sg / neff_name
            shutil.copy(hit, out)
            return str(out)
        out = orig(bir_json, tmpdir, neff_name)
        try:
            shutil.copy(out, hit)
        except OSError:
            pass
        return out

    b2j.compile_bir_kernel = cached
    b2j._ant_neff_cache_installed = True


def _run(in_maps, trace=False):
    _install_neff_cache()
    nc = _get_nc()
    return run_bass_kernel_spmd(nc, in_maps, core_ids=list(range(B)), trace=trace)


def kernel(x, Wq, bq, Wk, bk, Wv, bv, gamma, _trace=False):
    x = np.asarray(x, dtype=np.float32)
    in_maps = _prep_inputs(x, Wq, bq, Wk, bk, Wv, bv, gamma)
    res = _run(in_maps, trace=_trace)
    out = np.stack([res.results[b]["out"] for b in range(B)])
    out = out.reshape(x.shape).astype(np.float32)
    if _trace:
        return out, res
    return out


def _enable_ntff_hook():
    """Register the axon NTFF profile hook (missing antenv.axon_hooks shim)."""
    import sys, types

    if "antenv.axon_hooks" in sys.modules:
        return
    mod = types.ModuleType("antenv.axon_hooks")
    mod._hook = None
    mod.set_axon_ntff_profile_hook = lambda h: setattr(mod, "_hook", h)
    mod.get_axon_ntff_profile_hook = lambda: mod._hook
    sys.modules["antenv.axon_hooks"] = mod
    import antenv

    antenv.axon_hooks = mod
    from trn_agent_boot.trn_boot import _ntff_profile_via_ctypes

    mod._hook = _ntff_profile_via_ctypes("/opt/axon/libaxon_pjrt.so")


# revision 20
# speedup vs baseline: 1.0387x; 1.0387x over previous
"""SAGAN-style attention block on 8 TRN2 NeuronCores, data-parallel over batch.

Per core (one batch b): x_b [C=256, N=4096] f32.
  q = Wq x + bq  [32, N];  k = Wk x + bk  [32, N]
  S = q^T k  [N, N];  attn = softmax(S, axis=0)  (column softmax over i)
  out = gamma * (v @ attn) + x,  v = Wv x + bv

Device algorithm (bf16 matmuls, f32 PSUM accumulation):
  - no max-subtraction in softmax: |S| < ~50 empirically, exp() fits bf16
  - bv folded into the residual: out = gamma*(v0@E)/s + (x + gamma*bv)
  - qk emitted as 4-way row-tiled waves (tile_position) into a [P,4,JT]
    PSUM tile; exp activations read FD=2048 (4 banks) at a time
  - colsum via 4-way col-tiled ones[128,32] matmuls reading E directly
    (partials in 4 partition groups), then one broadcast matmul with a
    1/(32*gamma) stationary -> s/gamma on all 128 partitions
  - v@E matmuls split into M=64 col-tile pairs so each LDWEIGHTS (64
    cols) can hide behind the concurrent matmul on the other col group
"""

import numpy as np
import ml_dtypes

import concourse.bass as bass
import concourse.mybir as mybir
from concourse import bacc, tile
from concourse.bass import ds
from concourse.bass_utils import run_bass_kernel_spmd

F32 = mybir.dt.float32
BF16 = mybir.dt.bfloat16
FP8 = mybir.dt.float8e4
AF = mybir.ActivationFunctionType
ALU = mybir.AluOpType

B, C, N = 8, 256, 4096
C8 = 32
P = 128
JT = 512          # j-tile width
NJT = N // JT     # 8 j-tiles
NKC = N // P      # 32 i/k chunks of 128

_cache = {}


def _build_nc():
    nc = bacc.Bacc("TRN2", target_bir_lowering=False, debug=False, num_devices=B)

    x16_d = nc.dram_tensor("x16", [C, N], BF16, kind="ExternalInput").ap()
    wq_d = nc.dram_tensor("wq", [P, 2, P], BF16, kind="ExternalInput").ap()
    wk_d = nc.dram_tensor("wk", [P, 2, P], BF16, kind="ExternalInput").ap()
    wvt_d = nc.dram_tensor("wvt", [P, 2, C], BF16, kind="ExternalInput").ap()
    bq_d = nc.dram_tensor("bq", [P, 1], F32, kind="ExternalInput").ap()
    bk_d = nc.dram_tensor("bk", [P, 1], F32, kind="ExternalInput").ap()
    gbv_d = nc.dram_tensor("gbv", [P, 2], F32, kind="ExternalInput").ap()
    bones_d = nc.dram_tensor("bones", [P, P], BF16, kind="ExternalInput").ap()
    out_d = nc.dram_tensor("out", [C, N], F32, kind="ExternalOutput").ap()

    out_view = out_d.rearrange("(o p) n -> p o n", p=P)
    x16_view = x16_d.rearrange("(o p) n -> p o n", p=P)

    with tile.TileContext(nc) as tc:
        with (
            tc.tile_pool(name="const", bufs=1) as cpool,
            tc.tile_pool(name="big", bufs=1) as bigpool,
            tc.tile_pool(name="epool", bufs=2) as epool,
            tc.tile_pool(name="work", bufs=2) as wpool,
            tc.tile_pool(name="psA", bufs=1, space="PSUM") as psA,
            tc.tile_pool(name="psU", bufs=2, space="PSUM") as psU,
            tc.tile_pool(name="psS", bufs=1, space="PSUM") as psS,
            tc.tile_pool(name="psR", bufs=1, space="PSUM") as psR,
        ):
            # ---- big input first: x16 (2 MB) is the ONLY big input DMA.
            # The residual is added in bf16 (rel rms ~1e-3, far under the
            # 2e-2 gate) with gamma*bv folded into the fused normalize op.
            # First chunks are small so the projections can start early.
            x16_sb = bigpool.tile([P, 2, N], BF16, tag="x16")
            x16_chunks = [512, 512, 1024, 1024, 1024]
            off = 0
            for w_ch in x16_chunks:
                nc.sync.dma_start(
                    out=x16_sb[:, :, ds(off, w_ch)],
                    in_=x16_view[:, :, ds(off, w_ch)],
                )
                off += w_ch

            # ---- constants ----
            wq_sb = cpool.tile([P, 2, P], BF16, tag="wq")
            nc.sync.dma_start(out=wq_sb[:], in_=wq_d)
            wk_sb = cpool.tile([P, 2, P], BF16, tag="wk")
            nc.sync.dma_start(out=wk_sb[:], in_=wk_d)
            bq_sb = cpool.tile([P, 1], F32, tag="bq")
            nc.sync.dma_start(out=bq_sb[:], in_=bq_d)
            bk_sb = cpool.tile([P, 1], F32, tag="bk")
            nc.sync.dma_start(out=bk_sb[:], in_=bk_d)
            wvt_sb = cpool.tile([P, 2, C], BF16, tag="wvt")
            nc.sync.dma_start(out=wvt_sb[:], in_=wvt_d)
            gbv_sb = cpool.tile([P, 2], F32, tag="gbv")
            nc.sync.dma_start(out=gbv_sb[:], in_=gbv_d)
            bones_sb = cpool.tile([P, P], BF16, tag="bones")
            nc.sync.dma_start(out=bones_sb[:], in_=bones_d)
            ones32_sb = cpool.tile([P, C8], BF16, tag="ones32")
            nc.vector.memset(ones32_sb[:], 1.0)

            # preload the exp table during the DMA wait
            dummy = cpool.tile([P, 1], BF16, tag="dummy")
            nc.scalar.activation(dummy[:], bq_sb[:], AF.Exp)

            q_sb = bigpool.tile([P, N], BF16, tag="q")
            k_sb = bigpool.tile([P, N], BF16, tag="k")
            vt_sb = bigpool.tile([P, NKC, C], BF16, tag="vt")

            # ---- q/k projections, chunk-major so qk wave 0 can start after
            # the first n-chunk. They run in the psS/psR banks (idle during
            # the projection phase) so superstep 0's qk waves never WAR-wait
            # the projection tail in the psA banks.
            # out[m, n] = sum_c W_rep[c, m] * x16[c, n]  (M=128: 4 replicas)
            for g in range(4):
                for gi, (w_sb, b_sb, dst) in enumerate(
                    ((wq_sb, bq_sb, q_sb), (wk_sb, bk_sb, k_sb))
                ):
                    for t2 in range(2):
                        nch = 2 * g + t2
                        pool = psS if (2 * gi + t2) % 2 == 0 else psR
                        tag = "s" if pool is psS else "r"
                        sl = pool.tile([P, JT], F32, tag=tag,
                                       name=f"proj_{nch}_{gi}")
                        for kc in range(2):
                            nc.tensor.matmul(
                                sl[:],
                                w_sb[:, kc, :],
                                x16_sb[:, kc, ds(nch * JT, JT)],
                                start=(kc == 0),
                                stop=(kc == 1),
                            )
                        nc.vector.tensor_scalar(
                            out=dst[:, ds(nch * JT, JT)],
                            in0=sl[:],
                            scalar1=b_sb[:, :],
                            scalar2=None,
                            op0=ALU.add,
                        )

            # vT projection emitted inside superstep 0 (interleaved below),
            # using the psU buffers (idle until superstep 1):
            # out[n, c] = sum_c' x16[c', n] * WvT[c', c]
            def emit_vt_group(g):
                sl = psU.tile([P, 2, C], F32, tag="u", name=f"vtp_{g}")
                for t2 in range(2):
                    nck = 2 * g + t2
                    for kc in range(2):
                        nc.tensor.matmul(
                            sl[:, t2, :],
                            x16_sb[:, kc, ds(nck * P, P)],
                            wvt_sb[:, kc, :],
                            start=(kc == 0),
                            stop=(kc == 1),
                        )
                nc.vector.tensor_copy(
                    out=vt_sb[:, ds(2 * g, 2), :], in_=sl[:]
                )

            # ---- main software-pipelined loop over j-tiles ----
            # Per superstep t: qk+exp waves for tile t, v@E for tile t-1.
            # Colsum pair-waves lag 2 waves behind their acts (so the DVE
            # pair-sum latency never stalls the TensorE queue); the last 4
            # pairs of tile t and its finalize (copy/bcast/recip) are spread
            # over waves 0-2 of superstep t+1.
            e_tiles = {}
            ep_tiles = {}
            eq_tiles = {}
            s_ps_t = {}
            r_tiles = {}
            u_ps_t = {}

            for step in range(NJT + 1):
                t_prev = step - 1

                if step < NJT:
                    e_tiles[step] = epool.tile(
                        [P, NKC, JT], BF16, tag="E", name=f"E_{step}"
                    )
                    ep_tiles[step] = epool.tile(
                        [P, NKC // 2, JT], BF16, tag="Ep", name=f"Ep_{step}",
                        bufs=1,
                    )
                    eq_tiles[step] = epool.tile(
                        [P, NJT, JT], BF16, tag="Eq", name=f"Eq_{step}"
                    )
                    s_ps_t[step] = psS.tile([P, JT], F32, tag="s", name=f"s_{step}")
                if step >= 1:
                    u_ps_t[t_prev] = [
                        psU.tile([P, JT], F32, tag="u", name=f"u_{t_prev}_{m}")
                        for m in range(2)
                    ]

                js = ds(step * JT, JT) if step < NJT else None
                e_cur = e_tiles.get(step)
                ep_cur = ep_tiles.get(step)
                e_prev = e_tiles.get(t_prev)
                qk_ps = (
                    psA.tile([P, 4, JT], F32, tag="qk", name=f"qk_{step}")
                    if step < NJT else None
                )

                def emit_cs_pairwave(t, wp):
                    # 1 col-tiled ones matmul on the quad sum of wave wp
                    cg = wp % 4
                    nc.tensor.matmul(
                        s_ps_t[t][ds(32 * cg, 32), :],
                        ones32_sb[:],
                        eq_tiles[t][:, wp, :],
                        start=(wp < 4),
                        stop=(wp >= NJT - 4),
                        tile_position=(0, 32 * cg),
                    )

                def emit_finalize_piece(t, w):
                    # spread tile t's colsum tail + finalize over waves 0-2
                    if w == 0:
                        emit_cs_pairwave(t, NJT - 2)
                    elif w == 1:
                        emit_cs_pairwave(t, NJT - 1)
                        s128 = wpool.tile([P, JT], BF16, tag="s128")
                        nc.vector.tensor_copy(out=s128[:], in_=s_ps_t[t][:])
                        s_ps_t[t] = s128
                    elif w == 2:
                        rps = psR.tile([P, JT], F32, tag="r")
                        nc.tensor.matmul(rps[:], bones_sb[:], s_ps_t[t][:],
                                         start=True, stop=True)
                        r_sb = wpool.tile([P, JT], F32, tag="r")
                        nc.vector.reciprocal_approx_fast(out=r_sb[:], in_=rps[:])
                        r_tiles[t] = r_sb

                for w in range(NJT):
                    # v@E slots for tile t-1 (8 slots = 16 M=64 matmuls)
                    if step >= 1:
                        for s in range(8):
                            # m-major: all m=0 chunks then m=1, so each u
                            # bank's final write lands mid-wave and the next
                            # superstep's start=True matmuls never stall on
                            # the normalize muls
                            m, kc = s // 4, 4 * w + s % 4
                            for h in range(2):
                                nc.tensor.matmul(
                                    u_ps_t[t_prev][m][ds(64 * h, 64), :],
                                    vt_sb[:, kc, ds(128 * m + 64 * h, 64)],
                                    e_prev[:, kc, :],
                                    start=(kc == 0),
                                    stop=(kc == NKC - 1),
                                    tile_position=(0, 64 * h),
                                )
                    # qk wave w: 4 row-tiled matmuls (rows cycle 0..3),
                    # directly after the v@E slots: the act(w-1) -> qk(w)
                    # -> act(w) chain is the pacing loop
                    if step < NJT:
                        for r in range(4):
                            ic = 4 * w + r
                            nc.tensor.matmul(
                                qk_ps[:, r, :],
                                q_sb[32 * r: 32 * (r + 1), ds(ic * P, P)],
                                k_sb[32 * r: 32 * (r + 1), js],
                                start=True,
                                stop=True,
                                tile_position=(32 * r, 0),
                            )
                    # previous tile's colsum tail + finalize (waves 0-2)
                    if step >= 1 and w <= 2:
                        emit_finalize_piece(t_prev, w)
                    # colsum pair-waves for tile t, lagging 2 waves
                    if step < NJT and w >= 2:
                        emit_cs_pairwave(step, w - 2)
                    # normalize tile t-1 late in the superstep: free the u
                    # banks (muls first), then residual-add and store
                    if step >= 1 and w == NJT - 1:
                        js_prev = ds(t_prev * JT, JT)
                        r_sb = r_tiles[t_prev]
                        outt = wpool.tile([P, 2, JT], F32, tag="outt")
                        tmps = []
                        for m in range(2):
                            tmp = wpool.tile([P, JT], F32, tag="tmp")
                            nc.vector.tensor_mul(
                                out=tmp[:], in0=u_ps_t[t_prev][m][:],
                                in1=r_sb[:],
                            )
                            tmps.append(tmp)
                        for m in range(2):
                            nc.vector.scalar_tensor_tensor(
                                out=outt[:, m, :],
                                in0=tmps[m][:],
                                scalar=gbv_sb[:, m, None],
                                in1=x16_sb[:, m, js_prev],
                                op0=ALU.add,
                                op1=ALU.add,
                            )
                        nc.sync.dma_start(
                            out=out_view[:, :, js_prev], in_=outt[:]
                        )

                    # activation over all 4 banks (FD=2048)
                    if step < NJT:
                        nc.scalar.activation(
                            e_cur[:, ds(4 * w, 4), :], qk_ps[:], AF.Exp
                        )
                        # chunk-pair + quad sums (DVE): quad w lands in
                        # ep[w] = sum of the wave's 4 chunks
                        nc.vector.tensor_add(
                            out=ep_cur[:, 2 * w, :],
                            in0=e_cur[:, 4 * w, :],
                            in1=e_cur[:, 4 * w + 1, :],
                        )
                        nc.vector.tensor_add(
                            out=ep_cur[:, 2 * w + 1, :],
                            in0=e_cur[:, 4 * w + 2, :],
                            in1=e_cur[:, 4 * w + 3, :],
                        )
                        nc.vector.tensor_add(
                            out=eq_tiles[step][:, w, :],
                            in0=ep_cur[:, 2 * w, :],
                            in1=ep_cur[:, 2 * w + 1, :],
                        )
                    # superstep 0 filler: vT projection
                    if step == 0:
                        emit_vt_group(2 * w)
                        emit_vt_group(2 * w + 1)
    nc.compile()
    return nc


def _prep_inputs(x, Wq, bq, Wk, bk, Wv, bv, gamma):
    x = np.asarray(x, dtype=np.float32)
    Wq = np.asarray(Wq, dtype=np.float32)
    bq = np.asarray(bq, dtype=np.float32)
    Wk = np.asarray(Wk, dtype=np.float32)
    bk = np.asarray(bk, dtype=np.float32)
    Wv = np.asarray(Wv, dtype=np.float32)
    bv = np.asarray(bv, dtype=np.float32)
    g = float(np.asarray(gamma))

    bf = ml_dtypes.bfloat16
    # WqT replicated 4x along M so q lands replicated across 4x32 partitions
    wq_rep = np.tile(Wq.T, (1, 4)).reshape(2, P, P).transpose(1, 0, 2)
    wk_rep = np.tile(Wk.T, (1, 4)).reshape(2, P, P).transpose(1, 0, 2)
    wvt = Wv.T.reshape(2, P, C).transpose(1, 0, 2)
    bq_rep = np.tile(bq, 4)[:, None].astype(np.float32)
    bk_rep = np.tile(bk, 4)[:, None].astype(np.float32)
    gbv = (g * bv).reshape(2, P).T.copy().astype(np.float32)
    inv32g = 1.0 / (32.0 * g) if g != 0.0 else 0.0
    bones = np.full((P, P), inv32g, dtype=np.float32)

    xf = x.reshape(B, C, N)
    x16 = xf.astype(bf)

    shared = {
        "wq": np.ascontiguousarray(wq_rep.astype(bf)),
        "wk": np.ascontiguousarray(wk_rep.astype(bf)),
        "wvt": np.ascontiguousarray(wvt.astype(bf)),
        "bq": bq_rep,
        "bk": bk_rep,
        "gbv": gbv,
        "bones": bones.astype(bf),
    }
    in_maps = []
    for b in range(B):
        m = dict(shared)
        m["x16"] = np.ascontiguousarray(x16[b])
        in_maps.append(m)
    return in_maps


def _get_nc():
    if "nc" not in _cache:
        _cache["nc"] = _build_nc()
    return _cache["nc"]


def _install_neff_cache():
    """Cache compiled NEFFs by BIR hash: the bass_exec path skips the
    regular neuron compile cache, costing ~10min of walrus per process."""
    import hashlib
    import pathlib
    import shutil

    from concourse import bass2jax as b2j

    if getattr(b2j, "_ant_neff_cache_installed", False):
        return
    orig = b2j.compile_bir_kernel
    cache_dir = pathlib.Path("/root/.neuron-compile-cache/bass_neff")
    try:
        cache_dir.mkdir(parents=True, exist_ok=True)
    except OSError:
        return

    def cached(bir_json, tmpdir, neff_name="file.neff"):
        raw = bir_json if isinstance(bir_json, bytes) else bir_json.encode()
        h = hashlib.sha256(raw).hexdigest()
        hit = cache_dir / f"{h}.neff"
        if hit.exists():
            sg = pathlib.Path(tmpdir) / "sg00"
            sg.mkdir(parents=True, exist_ok=True)
            out = sg / neff_name
            shutil.copy(hit, out)
            return str(out)
        out = orig(bir_json, tmpdir, neff_name)
        try:
            shutil.copy(out, hit)
        except OSError:
            pass
        return out

    b2j.compile_bir_kernel = cached
    b2j._ant_neff_cache_installed = True


def _run(in_maps, trace=False):
    _install_neff_cache()
    nc = _get_nc()
    return run_bass_kernel_spmd(nc, in_maps, core_ids=list(range(B)), trace=trace)


def kernel(x, Wq, bq, Wk, bk, Wv, bv, gamma, _trace=False):
    x = np.asarray(x, dtype=np.float32)
    in_maps = _prep_inputs(x, Wq, bq, Wk, bk, Wv, bv, gamma)
    res = _run(in_maps, trace=_trace)
    out = np.stack([res.results[b]["out"] for b in range(B)])
    out = out.reshape(x.shape).astype(np.float32)
    if _trace:
        return out, res
    return out


def _enable_ntff_hook():
    """Register the axon NTFF profile hook (missing antenv.axon_hooks shim)."""
    import sys, types

    if "antenv.axon_hooks" in sys.modules:
        return
    mod = types.ModuleType("antenv.axon_hooks")
    mod._hook = None
    mod.set_axon_ntff_profile_hook = lambda h: setattr(mod, "_hook", h)
    mod.get_axon_ntff_profile_hook = lambda: mod._hook
    sys.modules["antenv.axon_hooks"] = mod
    import antenv

    antenv.axon_hooks = mod
    from trn_agent_boot.trn_boot import _ntff_profile_via_ctypes

    mod._hook = _ntff_profile_via_ctypes("/opt/axon/libaxon_pjrt.so")
